# revision 18
# baseline (speedup 1.0000x reference)
"""Trainium2 Bass kernel for nn_Attention_81956565942967.

Cross-attention with key-length masking:
  B=8, N=1024, DIM=1024, HEADS=16, DIM_HEAD=64.

Sharding: head-pair tensor parallel. Core i owns inner dims
[128*i, 128*i+128) (head pair i) of Wq/Wkv (columns) and Wout (rows),
and processes ALL 8 batches for that pair — but only the
ceil(lengths[b]/128) valid key chunks of each batch (sum over the
seed-0 lengths: 40 of 64 chunks). Every core runs the same
instruction stream (shapes depend only on lengths, shared across
cores); only the weight-slice DATA differs per core, so the program is
SPMD-uniform yet perfectly load balanced despite the skewed lengths.
Each core emits 8 partial outputs pout_b = catT_pair^T @ Wout_pair in
bf16; the host sums the 8 cores' partials and adds the bias.

Device algorithm per batch b (c = ceil(len_b/128) key chunks):
  qT = Wq_p^T xT        [128 pair dims, 1024 queries]
  kT = Wk_p^T cT        [128, 128c]
  v  = cT^T Wv_p        [128c keys, 2, 65] with mask col at 64
  dotsT[j,i] = kT_h^T qT_h per (i-block 512, sub-head), exp on ACT
  po[i, (v|rs)] = sum_j expT^T [v | mask]; normalize on DVE -> cat
  catT = DMA-xbar-transpose(cat)  (sync queue, one batch delayed)
  pout = catT^T Wout_p  (single-matmul psum tiles, K=128)

Schedule (3-deep software pipeline over batches, heavy batches first):
iteration for batch b emits
  [transposes of b-1] [dots(b) x fg(b-2) interleave] [q,k proj b+1]
  [av(b) x v-proj(b+1) interleave] [input DMA b+2]
so the sync queue (transposes+inputs) and scalar queue (exp + fg
casts) never carry an op whose dependency isn't already complete —
head-of-line blocking on the DMA queues was the main stall source.
out-proj casts run on ACT (Copy) to unload DVE; out DMAs on gpsimd.
"""

from contextlib import ExitStack

import ml_dtypes
import numpy as np

import concourse.bass as bass
from concourse import bacc
import concourse.mybir as mybir
import concourse.tile as tile
from concourse.bass_utils import run_bass_kernel_spmd

B, N, DIM = 8, 1024, 1024
HEADS, DIM_HEAD = 16, 64
INNER = HEADS * DIM_HEAD
SCALE = DIM_HEAD ** -0.5

P = 128
KT = DIM // P    # 8 contraction tiles along dim
FI = 512         # free-dim tile (PSUM bank)
NI = N // FI     # 2 query blocks
VW = DIM_HEAD + 1  # 65: v block per sub-head + mask col (rowsum)

BF = mybir.dt.bfloat16
F32 = mybir.dt.float32

_CACHE: dict = {}


def _build(cs: tuple) -> bass.Bass:
    """cs[b] = number of 128-key chunks for batch b; program shape
    depends only on cs (same on all cores)."""
    nc = bacc.Bacc("TRN2")
    order = sorted(range(B), key=lambda b: -cs[b])  # heavy batches first
    ctot = sum(cs)
    coff = np.cumsum([0] + list(cs))  # mask column offset per batch

    xT_d = [nc.dram_tensor(f"xT{b}", [DIM, N], BF, kind="ExternalInput").ap()
            for b in range(B)]
    cT_d = [nc.dram_tensor(f"cT{b}", [DIM, P * cs[b]], BF,
                           kind="ExternalInput").ap() for b in range(B)]
    wq_d = nc.dram_tensor("wq_p", [DIM, P], BF, kind="ExternalInput").ap()
    wk_d = nc.dram_tensor("wk_p", [DIM, P], BF, kind="ExternalInput").ap()
    wv_d = nc.dram_tensor("wv_p", [DIM, P], BF, kind="ExternalInput").ap()
    wout_d = nc.dram_tensor("wout_p", [P, DIM], BF, kind="ExternalInput").ap()
    mask_d = nc.dram_tensor("maskb", [P, ctot], F32, kind="ExternalInput").ap()
    ident_d = nc.dram_tensor("ident", [P, P], F32, kind="ExternalInput").ap()
    out_d = [nc.dram_tensor(f"pout{b}", [N, DIM], BF,
                            kind="ExternalOutput").ap() for b in range(B)]

    with tile.TileContext(nc) as tc, ExitStack() as ctx:
        const_p = ctx.enter_context(tc.tile_pool(name="const", bufs=1))
        roll_p = ctx.enter_context(tc.tile_pool(name="roll", bufs=2))
        exp_p = ctx.enter_context(tc.tile_pool(name="expp", bufs=2))
        stage_p = ctx.enter_context(tc.tile_pool(name="stage", bufs=4))
        acc_ps = ctx.enter_context(tc.tile_pool(name="accps", bufs=2, space="PSUM"))
        dots_ps = ctx.enter_context(tc.tile_pool(name="dotsps", bufs=2, space="PSUM"))
        po_ps = ctx.enter_context(tc.tile_pool(name="pops", bufs=2, space="PSUM"))

        # --- static SBUF tensors ---
        wq_sb = const_p.tile([P, KT, P], BF, tag="wq")
        wk_sb = const_p.tile([P, KT, P], BF, tag="wk")
        wv_sb = const_p.tile([P, KT, P], BF, tag="wv")
        wout_sb = const_p.tile([P, DIM], BF, tag="wout")
        mask_sb = const_p.tile([P, ctot], F32, tag="mask")
        ident_sb = const_p.tile([P, P], F32, tag="ident")

        # --- DMA emission helpers (sync queue: inputs, in batch order) ---
        def dma_x(b, slot):
            for k in range(KT):
                nc.sync.dma_start(out=slot[:, k, :],
                                  in_=xT_d[b][k * P:(k + 1) * P, :])

        def dma_c(b, slot):
            w = P * cs[b]
            for k in range(KT):
                nc.sync.dma_start(out=slot[:, k, 0:w],
                                  in_=cT_d[b][k * P:(k + 1) * P, :])

        # --- compute helpers ---
        def proj_q(b, xs, qslot, i):
            ps = acc_ps.tile([P, FI], F32, tag="acc", name="ps")
            for k in range(KT):
                nc.tensor.matmul(
                    ps, wq_sb[:, k, :], xs[:, k, i * FI:(i + 1) * FI],
                    start=(k == 0), stop=(k == KT - 1),
                )
            nc.vector.tensor_copy(qslot[:, i * FI:(i + 1) * FI], ps)

        def proj_k(b, cls, kslot, t0):
            w = P * cs[b]
            t1 = min(t0 + FI, w)
            ps = acc_ps.tile([P, FI], F32, tag="acc", name="ps")
            for k in range(KT):
                nc.tensor.matmul(
                    ps[:, 0:t1 - t0], wk_sb[:, k, :], cls[:, k, t0:t1],
                    start=(k == 0), stop=(k == KT - 1),
                )
            nc.vector.tensor_copy(kslot[:, t0:t1], ps[:, 0:t1 - t0])

        def proj_v(b, cls, vslot, j):
            # v natural layout [128 keys, 2 sub-heads, 65] per chunk j
            psw = acc_ps.tile([P, FI], F32, tag="acc", name="psv")
            ps = psw[:, 0:P]
            for k in range(KT):
                nc.tensor.matmul(
                    ps, cls[:, k, j * P:(j + 1) * P], wv_sb[:, k, :],
                    start=(k == 0), stop=(k == KT - 1),
                )
            v3 = vslot[:, j, :].rearrange("p (s w) -> p s w", w=VW)
            nc.vector.tensor_copy(
                v3[:, :, 0:DIM_HEAD],
                ps.rearrange("p (s d) -> p s d", d=DIM_HEAD),
            )
            vj = vslot[:, j, :]
            diag = bass.AP(tensor=vj.tensor, offset=vj.offset + DIM_HEAD,
                           ap=[list(vj.ap[0]), [VW, 2]])
            nc.vector.tensor_scalar_mul(
                diag,
                mask_sb[:, coff[b] + j:coff[b] + j + 1].to_broadcast([P, 2]),
                1.0)

        ets_of = {}

        def dots_exp(b, i, qslot, kslot, fillers):
            c = cs[b]
            isl = slice(i * FI, (i + 1) * FI)
            ets = [exp_p.tile([P, 8, FI], BF, tag=f"exp{sub}", name=f"et{sub}")
                   for sub in range(2)]
            ets_of[(b, i)] = ets
            j0 = 0
            while j0 < c:
                jn = min(2, c - j0)
                dpss = [dots_ps.tile([P, 2, FI], F32, tag="dots", name="dps")
                        for _ in range(2)]
                for jj in range(jn):
                    j = j0 + jj
                    for sub in range(2):
                        off = sub * DIM_HEAD
                        nc.tensor.matmul(
                            dpss[sub][:, jj, :],
                            kslot[off:off + DIM_HEAD, j * P:(j + 1) * P],
                            qslot[off:off + DIM_HEAD, isl],
                            start=True, stop=True,
                        )
                for sub in range(2):
                    nc.scalar.activation(
                        ets[sub][:, j0:j0 + jn, :], dpss[sub][:, 0:jn, :],
                        mybir.ActivationFunctionType.Exp, scale=SCALE,
                    )
                j0 += jn
                # PE filler between dots groups (out-proj of b-2, or
                # q/k chains of the next batch on the first iteration)
                for _ in range(2):
                    if fillers:
                        fillers.pop(0)()

        def av_norm(b, g, vslot, catslot, catTslot):
            # query tile g (128 queries) of batch b; i-block = g // 4
            c = cs[b]
            i = g // (FI // P)
            it = g % (FI // P)
            ets = ets_of[(b, i)]
            po = po_ps.tile([P, 2, VW], F32, tag="po", name="po")
            for sub in range(2):
                et = ets[sub]
                for j in range(c):
                    nc.tensor.matmul(
                        po[:, sub, :],
                        et[:, j, it * P:(it + 1) * P],
                        vslot[:, j, :].rearrange(
                            "p (s w) -> p s w", w=VW)[:, sub, :],
                        start=(j == 0), stop=(j == c - 1),
                    )
            rr = stage_p.tile([P, 2], F32, tag="rr", name="rr", bufs=8)
            nc.vector.reciprocal(rr, po[:, :, DIM_HEAD])
            cat3 = catslot[:, g, :].rearrange("p (s d) -> p s d", d=DIM_HEAD)
            nc.vector.tensor_tensor(
                cat3, po[:, :, 0:DIM_HEAD],
                rr.rearrange("p (s o) -> p s o", o=1).to_broadcast(
                    [P, 2, DIM_HEAD]),
                mybir.AluOpType.mult,
            )

        def transpose_g(catslot, catTslot, g):
            # PE transpose (~110ns, fp32 so it can share the acc psum
            # ring) + small DVE copy-with-cast, instead of the 1.2us
            # DMA-xbar transpose that clogged the HWDGE queues
            tpw = acc_ps.tile([P, FI], F32, tag="acc", name="tp")
            tp = tpw[:, 0:P]
            nc.tensor.transpose(tp, catslot[:, g, :], ident_sb)
            nc.scalar.activation(catTslot[:, g * P:(g + 1) * P], tp,
                                 mybir.ActivationFunctionType.Copy)

        def final_group(b, t, i2, catTslot, on_act=False):
            pf = acc_ps.tile([P, FI], F32, tag="acc", name="pf")
            nc.tensor.matmul(
                pf, catTslot[:, t * P:(t + 1) * P],
                wout_sb[:, i2 * FI:(i2 + 1) * FI],
                start=True, stop=True,
            )
            ot = stage_p.tile([P, FI], BF, tag="ot", name="ot", bufs=10)
            # psum->bf16 drain on DVE inside dots regions (ACT is busy
            # with exp there); out DMAs alternate sync HWDGE / gpsimd
            if on_act:
                nc.scalar.activation(ot, pf, mybir.ActivationFunctionType.Copy)
            else:
                nc.vector.tensor_copy(ot, pf)
            if (t + i2) % 2 == 0:
                nc.sync.dma_start(
                    out=out_d[b][t * P:(t + 1) * P, i2 * FI:(i2 + 1) * FI],
                    in_=ot)
            else:
                nc.gpsimd.dma_start(
                    out=out_d[b][t * P:(t + 1) * P, i2 * FI:(i2 + 1) * FI],
                    in_=ot)

        # --- slot allocation per batch (rolling pools) ---
        def alloc_x():
            return roll_p.tile([P, KT, N], BF, tag="xslot", name="xs")

        def alloc_c():
            return roll_p.tile([P, KT, N], BF, tag="cslot", name="cls")

        # --- DMA preamble: weights + first batch, chunk-interleaved ---
        b0 = order[0]
        st = {}
        xs0 = alloc_x()
        for k in range(KT):
            nc.sync.dma_start(out=wq_sb[:, k, :],
                                in_=wq_d[k * P:(k + 1) * P, :])
            nc.sync.dma_start(out=xs0[:, k, :],
                                in_=xT_d[b0][k * P:(k + 1) * P, :])
        cls0 = alloc_c()
        for k in range(KT):
            nc.sync.dma_start(out=wk_sb[:, k, :],
                                in_=wk_d[k * P:(k + 1) * P, :])
            nc.sync.dma_start(out=wv_sb[:, k, :],
                                in_=wv_d[k * P:(k + 1) * P, :])
            nc.sync.dma_start(out=cls0[:, k, 0:P * cs[b0]],
                                in_=cT_d[b0][k * P:(k + 1) * P, :])
        nc.sync.dma_start(out=mask_sb, in_=mask_d)
        nc.sync.dma_start(out=ident_sb, in_=ident_d)
        nc.sync.dma_start(out=wout_sb, in_=wout_d)
        st[b0] = {"x": xs0, "c": cls0}

        def stage_qkv_qk(b):
            q = roll_p.tile([P, N], BF, tag="qslot", name="qs")
            k = roll_p.tile([P, N], BF, tag="kslot", name="ks")
            st[b].update(q=q, k=k)
            for i in range(NI):
                proj_q(b, st[b]["x"], q, i)
            for t0 in range(0, P * cs[b], FI):
                proj_k(b, st[b]["c"], k, t0)

        def alloc_v(b):
            v = roll_p.tile([P, 8, 2 * VW], BF, tag="vslot", name="vs")
            st[b]["v"] = v
            return v

        def stage_dma(b):
            st[b] = {"x": alloc_x(), "c": alloc_c()}
            dma_x(b, st[b]["x"])
            dma_c(b, st[b]["c"])

        # Prologue: first batch projections + second batch DMA.
        stage_qkv_qk(b0)
        alloc_v(b0)
        for j in range(cs[b0]):
            proj_v(b0, st[b0]["c"], st[b0]["v"], j)
        if B > 1:
            stage_dma(order[1])

        for pidx, b in enumerate(order):
            bn = order[pidx + 1] if pidx + 1 < B else None
            bp = order[pidx - 1] if pidx >= 1 else None
            bpp = order[pidx - 2] if pidx >= 2 else None
            bnn = order[pidx + 2] if pidx + 2 < B else None
            cat = roll_p.tile([P, 8, P], F32, tag="catslot", name="cat",
                              bufs=3)
            catT = roll_p.tile([P, N], BF, tag="catTslot", name="catT",
                               bufs=3)
            st[b].update(cat=cat, catT=catT)

            # 0. transposes of the previous batch (deps complete -> the
            # sync queue never blocks input DMAs behind them)
            if bp is not None:
                for g in range(8):
                    transpose_g(st[bp]["cat"], st[bp]["catT"], g)

            # 1. dots of b, interleaved with out-proj of b-2 (or, on the
            # first iteration, with q/k chains of the next batch)
            if bpp is not None:
                fillers = [
                    (lambda t=t, i2=i2, bb=bpp, ct=st[bpp]["catT"]:
                     final_group(bb, t, i2, ct))
                    for t in range(8) for i2 in range(NI)]
                qk_as_filler = False
            elif bn is not None:
                # first two iterations have no out-proj backlog: use the
                # next batch's q/k chains as the dots-region PE filler
                q = roll_p.tile([P, N], BF, tag="qslot", name="qs")
                kk = roll_p.tile([P, N], BF, tag="kslot", name="ks")
                st[bn].update(q=q, k=kk)
                fillers = [
                    (lambda i=i: proj_q(bn, st[bn]["x"], q, i))
                    for i in range(NI)] + [
                    (lambda t0=t0: proj_k(bn, st[bn]["c"], kk, t0))
                    for t0 in range(0, P * cs[bn], FI)]
                qk_as_filler = True
            else:
                fillers = []
                qk_as_filler = False
            dots_exp(b, 0, st[b]["q"], st[b]["k"], fillers)
            dots_exp(b, 1, st[b]["q"], st[b]["k"], fillers)

            # 2. q/k projections of the next batch
            if bn is not None and not qk_as_filler:
                stage_qkv_qk(bn)

            # 3. av of b interleaved with v-proj of the next batch and
            # any dots-region filler leftovers (small-c batches)
            if bn is not None:
                alloc_v(bn)
            last = pidx == B - 1
            for g in range(8):
                av_norm(b, g, st[b]["v"], cat, catT)
                if bn is not None and g < cs[bn]:
                    proj_v(bn, st[bn]["c"], st[bn]["v"], g)
                if fillers:
                    fillers.pop(0)()
                if last:
                    # fold the final two batches' out-proj into the last
                    # av region: transpose each cat tile as soon as its
                    # norm lands, then emit its out-proj groups
                    blp = order[-2]
                    transpose_g(cat, catT, g)
                    final_group(blp, g, 0, st[blp]["catT"], on_act=False)
                    final_group(blp, g, 1, st[blp]["catT"], on_act=True)
            while fillers:
                fillers.pop(0)()

            # 4. input DMAs for batch b+2
            if bnn is not None:
                stage_dma(bnn)
            if bpp is not None:
                st.pop(bpp)

        # Epilogue: out-proj of the last batch (catT transposed per-g
        # inside the last av region above).
        bl = order[-1]
        for t in range(8):
            for i2 in range(NI):
                final_group(bl, t, i2, st[bl]["catT"], on_act=(i2 == 1))

    nc.finalize()
    return nc


def _prep_shared(x, context, lengths, Wq, Wkv, Wout, bout):
    """Host-side prep shared across cores."""
    bf = ml_dtypes.bfloat16
    cs = tuple(int(min(N, (int(l) + P - 1) // P * P) // P) for l in lengths)
    shared = {}
    for b in range(B):
        shared[f"xT{b}"] = np.ascontiguousarray(
            np.asarray(x[b]).T, dtype=bf)
        w = P * cs[b]
        cb = np.asarray(context[b][:w]).copy()
        cb[int(lengths[b]):] = 0.0
        shared[f"cT{b}"] = np.ascontiguousarray(cb.T, dtype=bf)
    # mask: column ct = chunk j of batch b; 1.0 where key row valid
    ctot = sum(cs)
    maskb = np.zeros((P, ctot), dtype=np.float32)
    col = 0
    for b in range(B):
        for j in range(cs[b]):
            rows = np.arange(j * P, (j + 1) * P)
            maskb[:, col] = (rows < int(lengths[b])).astype(np.float32)
            col += 1
    shared["maskb"] = np.ascontiguousarray(maskb)
    shared["ident"] = np.ascontiguousarray(np.eye(P, dtype=np.float32))
    return cs, shared


def _prep_in_maps(x, context, lengths, Wq, Wkv, Wout, bout):
    bf = ml_dtypes.bfloat16
    cs, shared = _prep_shared(x, context, lengths, Wq, Wkv, Wout, bout)
    wq = np.asarray(Wq)
    wkv = np.asarray(Wkv)
    wout = np.asarray(Wout)
    in_maps = []
    for core in range(B):
        s0, s1 = core * P, (core + 1) * P
        m = dict(shared)
        m["wq_p"] = np.ascontiguousarray(wq[:, s0:s1], dtype=bf)
        m["wk_p"] = np.ascontiguousarray(wkv[:, s0:s1], dtype=bf)
        m["wv_p"] = np.ascontiguousarray(wkv[:, INNER + s0:INNER + s1], dtype=bf)
        m["wout_p"] = np.ascontiguousarray(wout[s0:s1, :], dtype=bf)
        in_maps.append(m)
    return cs, in_maps


def run(inputs: dict, trace: bool = False):
    cs, in_maps = _prep_in_maps(**inputs)
    if cs not in _CACHE:
        _CACHE[cs] = _build(cs)
    nc = _CACHE[cs]
    res = run_bass_kernel_spmd(nc, in_maps, core_ids=list(range(B)), trace=trace)
    bout = np.asarray(inputs["bout"], dtype=np.float32)
    out = np.empty((B, N, DIM), dtype=np.float32)
    for b in range(B):
        acc = np.zeros((N, DIM), dtype=np.float32)
        for core in range(B):
            acc += np.asarray(res.results[core][f"pout{b}"],
                              dtype=np.float32)
        out[b] = acc + bout
    return out, res


def kernel(**inputs) -> np.ndarray:
    out, _ = run(inputs, trace=False)
    return out


# revision 19
# speedup vs baseline: 1.1200x; 1.1200x over previous
"""Trainium2 Bass kernel for nn_Attention_81956565942967.

Cross-attention with key-length masking:
  B=8, N=1024, DIM=1024, HEADS=16, DIM_HEAD=64.

Sharding: head-pair tensor parallel. Core i owns inner dims
[128*i, 128*i+128) (head pair i) of Wq/Wkv (columns) and Wout (rows),
and processes ALL 8 batches for that pair — but only the
ceil(lengths[b]/128) valid key chunks of each batch (sum over the
seed-0 lengths: 40 of 64 chunks). Every core runs the same
instruction stream (shapes depend only on lengths, shared across
cores); only the weight-slice DATA differs per core, so the program is
SPMD-uniform yet perfectly load balanced despite the skewed lengths.
Each core emits 8 partial outputs pout_b = catT_pair^T @ Wout_pair in
bf16; the host sums the 8 cores' partials and adds the bias.

Device algorithm per batch b (c = ceil(len_b/128) key chunks):
  qT = Wq_p^T xT        [128 pair dims, 1024 queries]
  kT = Wk_p^T cT        [128, 128c]
  v  = cT^T Wv_p        [128c keys, 2, 65] with mask col at 64
  dotsT[j,i] = kT_h^T qT_h per (i-block 512, sub-head), exp on ACT
  po[i, (v|rs)] = sum_j expT^T [v | mask]; normalize on DVE -> cat
  catT = DMA-xbar-transpose(cat)  (sync queue, one batch delayed)
  pout = catT^T Wout_p  (single-matmul psum tiles, K=128)

Schedule (3-deep software pipeline over batches, heavy batches first):
iteration for batch b emits
  [transposes of b-1] [dots(b) x fg(b-2) interleave] [q,k proj b+1]
  [av(b) x v-proj(b+1) interleave] [input DMA b+2]
so the sync queue (transposes+inputs) and scalar queue (exp + fg
casts) never carry an op whose dependency isn't already complete —
head-of-line blocking on the DMA queues was the main stall source.
out-proj casts run on ACT (Copy) to unload DVE; out DMAs on gpsimd.
"""

from contextlib import ExitStack

import ml_dtypes
import numpy as np

import concourse.bass as bass
from concourse import bacc
import concourse.mybir as mybir
import concourse.tile as tile
from concourse.bass_utils import run_bass_kernel_spmd

B, N, DIM = 8, 1024, 1024
HEADS, DIM_HEAD = 16, 64
INNER = HEADS * DIM_HEAD
SCALE = DIM_HEAD ** -0.5

P = 128
KT = DIM // P    # 8 contraction tiles along dim
FI = 512         # free-dim tile (PSUM bank)
NI = N // FI     # 2 query blocks
VW = DIM_HEAD + 1  # 65: v block per sub-head + mask col (rowsum)

BF = mybir.dt.bfloat16
F32 = mybir.dt.float32

_CACHE: dict = {}


def _build(cs: tuple) -> bass.Bass:
    """cs[b] = number of 128-key chunks for batch b; program shape
    depends only on cs (same on all cores)."""
    nc = bacc.Bacc("TRN2")
    order = sorted(range(B), key=lambda b: -cs[b])  # heavy batches first
    ctot = sum(cs)
    coff = np.cumsum([0] + list(cs))  # mask column offset per batch

    xT_d = [nc.dram_tensor(f"xT{b}", [DIM, N], BF, kind="ExternalInput").ap()
            for b in range(B)]
    cT_d = [nc.dram_tensor(f"cT{b}", [DIM, P * cs[b]], BF,
                           kind="ExternalInput").ap() for b in range(B)]
    wq_d = nc.dram_tensor("wq_p", [DIM, P], BF, kind="ExternalInput").ap()
    wk_d = nc.dram_tensor("wk_p", [DIM, P], BF, kind="ExternalInput").ap()
    wv_d = nc.dram_tensor("wv_p", [DIM, P], BF, kind="ExternalInput").ap()
    wout_d = nc.dram_tensor("wout_p", [P, DIM], BF, kind="ExternalInput").ap()
    mask_d = nc.dram_tensor("maskb", [P, ctot], F32, kind="ExternalInput").ap()
    ident_d = nc.dram_tensor("ident", [P, P], F32, kind="ExternalInput").ap()
    out_d = [nc.dram_tensor(f"pout{b}", [N, DIM], BF,
                            kind="ExternalOutput").ap() for b in range(B)]

    with tile.TileContext(nc) as tc, ExitStack() as ctx:
        const_p = ctx.enter_context(tc.tile_pool(name="const", bufs=1))
        roll_p = ctx.enter_context(tc.tile_pool(name="roll", bufs=2))
        exp_p = ctx.enter_context(tc.tile_pool(name="expp", bufs=2))
        stage_p = ctx.enter_context(tc.tile_pool(name="stage", bufs=4))
        acc_ps = ctx.enter_context(tc.tile_pool(name="accps", bufs=2, space="PSUM"))
        dots_ps = ctx.enter_context(tc.tile_pool(name="dotsps", bufs=2, space="PSUM"))
        po_ps = ctx.enter_context(tc.tile_pool(name="pops", bufs=2, space="PSUM"))

        # --- static SBUF tensors ---
        wq_sb = const_p.tile([P, KT, P], BF, tag="wq")
        wk_sb = const_p.tile([P, KT, P], BF, tag="wk")
        wv_sb = const_p.tile([P, KT, P], BF, tag="wv")
        wout_sb = const_p.tile([P, DIM], BF, tag="wout")
        mask_sb = const_p.tile([P, ctot], F32, tag="mask")
        ident_sb = const_p.tile([P, P], F32, tag="ident")

        # --- DMA emission helpers (sync queue: inputs, in batch order) ---
        def dma_x(b, slot):
            for k in range(KT):
                nc.sync.dma_start(out=slot[:, k, :],
                                  in_=xT_d[b][k * P:(k + 1) * P, :])

        def dma_c(b, slot):
            w = P * cs[b]
            for k in range(KT):
                nc.sync.dma_start(out=slot[:, k, 0:w],
                                  in_=cT_d[b][k * P:(k + 1) * P, :])

        # --- compute helpers ---
        def proj_q(b, xs, qslot, i):
            ps = acc_ps.tile([P, FI], F32, tag="acc", name="ps")
            for k in range(KT):
                nc.tensor.matmul(
                    ps, wq_sb[:, k, :], xs[:, k, i * FI:(i + 1) * FI],
                    start=(k == 0), stop=(k == KT - 1),
                )
            nc.vector.tensor_copy(qslot[:, i * FI:(i + 1) * FI], ps)

        def proj_k(b, cls, kslot, t0):
            w = P * cs[b]
            t1 = min(t0 + FI, w)
            ps = acc_ps.tile([P, FI], F32, tag="acc", name="ps")
            for k in range(KT):
                nc.tensor.matmul(
                    ps[:, 0:t1 - t0], wk_sb[:, k, :], cls[:, k, t0:t1],
                    start=(k == 0), stop=(k == KT - 1),
                )
            nc.vector.tensor_copy(kslot[:, t0:t1], ps[:, 0:t1 - t0])

        def proj_v(b, cls, vslot, j):
            # v natural layout [128 keys, 2 sub-heads, 65] per chunk j
            psw = acc_ps.tile([P, FI], F32, tag="acc", name="psv")
            ps = psw[:, 0:P]
            for k in range(KT):
                nc.tensor.matmul(
                    ps, cls[:, k, j * P:(j + 1) * P], wv_sb[:, k, :],
                    start=(k == 0), stop=(k == KT - 1),
                )
            v3 = vslot[:, j, :].rearrange("p (s w) -> p s w", w=VW)
            nc.vector.tensor_copy(
                v3[:, :, 0:DIM_HEAD],
                ps.rearrange("p (s d) -> p s d", d=DIM_HEAD),
            )
            vj = vslot[:, j, :]
            diag = bass.AP(tensor=vj.tensor, offset=vj.offset + DIM_HEAD,
                           ap=[list(vj.ap[0]), [VW, 2]])
            nc.vector.tensor_scalar_mul(
                diag,
                mask_sb[:, coff[b] + j:coff[b] + j + 1].to_broadcast([P, 2]),
                1.0)

        ets_of = {}

        def dots_exp(b, i, qslot, kslot, fillers):
            c = cs[b]
            isl = slice(i * FI, (i + 1) * FI)
            ets = [exp_p.tile([P, 8, FI], BF, tag=f"exp{sub}", name=f"et{sub}")
                   for sub in range(2)]
            ets_of[(b, i)] = ets
            j0 = 0
            while j0 < c:
                jn = min(2, c - j0)
                dpss = [dots_ps.tile([P, 2, FI], F32, tag="dots", name="dps")
                        for _ in range(2)]
                for jj in range(jn):
                    j = j0 + jj
                    for sub in range(2):
                        off = sub * DIM_HEAD
                        nc.tensor.matmul(
                            dpss[sub][:, jj, :],
                            kslot[off:off + DIM_HEAD, j * P:(j + 1) * P],
                            qslot[off:off + DIM_HEAD, isl],
                            start=True, stop=True,
                        )
                for sub in range(2):
                    nc.scalar.activation(
                        ets[sub][:, j0:j0 + jn, :], dpss[sub][:, 0:jn, :],
                        mybir.ActivationFunctionType.Exp, scale=SCALE,
                    )
                j0 += jn
                # PE filler between dots groups (out-proj of b-2, or
                # q/k chains of the next batch on the first iteration)
                for _ in range(2):
                    if fillers:
                        fillers.pop(0)()

        def av_norm(b, g, vslot, catslot, catTslot):
            # query tile g (128 queries) of batch b; i-block = g // 4
            c = cs[b]
            i = g // (FI // P)
            it = g % (FI // P)
            ets = ets_of[(b, i)]
            po = po_ps.tile([P, 2, VW], F32, tag="po", name="po")
            for sub in range(2):
                et = ets[sub]
                for j in range(c):
                    nc.tensor.matmul(
                        po[:, sub, :],
                        et[:, j, it * P:(it + 1) * P],
                        vslot[:, j, :].rearrange(
                            "p (s w) -> p s w", w=VW)[:, sub, :],
                        start=(j == 0), stop=(j == c - 1),
                    )
            rr = stage_p.tile([P, 2], F32, tag="rr", name="rr", bufs=8)
            nc.vector.reciprocal(rr, po[:, :, DIM_HEAD])
            cat3 = catslot[:, g, :].rearrange("p (s d) -> p s d", d=DIM_HEAD)
            nc.vector.tensor_tensor(
                cat3, po[:, :, 0:DIM_HEAD],
                rr.rearrange("p (s o) -> p s o", o=1).to_broadcast(
                    [P, 2, DIM_HEAD]),
                mybir.AluOpType.mult,
            )

        def transpose_g(catslot, catTslot, g):
            # PE transpose (~110ns, fp32 so it can share the acc psum
            # ring) + small DVE copy-with-cast, instead of the 1.2us
            # DMA-xbar transpose that clogged the HWDGE queues
            tpw = acc_ps.tile([P, FI], F32, tag="acc", name="tp")
            tp = tpw[:, 0:P]
            nc.tensor.transpose(tp, catslot[:, g, :], ident_sb)
            nc.vector.tensor_copy(catTslot[:, g * P:(g + 1) * P], tp)

        def final_group(b, t, i2, catTslot, on_act=False):
            pf = acc_ps.tile([P, FI], F32, tag="acc", name="pf")
            nc.tensor.matmul(
                pf, catTslot[:, t * P:(t + 1) * P],
                wout_sb[:, i2 * FI:(i2 + 1) * FI],
                start=True, stop=True,
            )
            ot = stage_p.tile([P, FI], BF, tag="ot", name="ot", bufs=10)
            # psum->bf16 drain on DVE inside dots regions (ACT is busy
            # with exp there); out DMAs alternate sync HWDGE / gpsimd
            if on_act:
                nc.scalar.activation(ot, pf, mybir.ActivationFunctionType.Copy)
            else:
                nc.vector.tensor_copy(ot, pf)
            if (t + i2) % 2 == 0:
                nc.sync.dma_start(
                    out=out_d[b][t * P:(t + 1) * P, i2 * FI:(i2 + 1) * FI],
                    in_=ot)
            else:
                nc.gpsimd.dma_start(
                    out=out_d[b][t * P:(t + 1) * P, i2 * FI:(i2 + 1) * FI],
                    in_=ot)

        # --- slot allocation per batch (rolling pools) ---
        def alloc_x():
            return roll_p.tile([P, KT, N], BF, tag="xslot", name="xs")

        def alloc_c():
            return roll_p.tile([P, KT, N], BF, tag="cslot", name="cls")

        # --- DMA preamble: weights + first batch, chunk-interleaved ---
        b0 = order[0]
        st = {}
        xs0 = alloc_x()
        for k in range(KT):
            nc.sync.dma_start(out=wq_sb[:, k, :],
                                in_=wq_d[k * P:(k + 1) * P, :])
            nc.sync.dma_start(out=xs0[:, k, :],
                                in_=xT_d[b0][k * P:(k + 1) * P, :])
        cls0 = alloc_c()
        for k in range(KT):
            nc.sync.dma_start(out=wk_sb[:, k, :],
                                in_=wk_d[k * P:(k + 1) * P, :])
            nc.sync.dma_start(out=wv_sb[:, k, :],
                                in_=wv_d[k * P:(k + 1) * P, :])
            nc.sync.dma_start(out=cls0[:, k, 0:P * cs[b0]],
                                in_=cT_d[b0][k * P:(k + 1) * P, :])
        nc.sync.dma_start(out=mask_sb, in_=mask_d)
        nc.sync.dma_start(out=ident_sb, in_=ident_d)
        nc.sync.dma_start(out=wout_sb, in_=wout_d)
        st[b0] = {"x": xs0, "c": cls0}

        def stage_qkv_qk(b):
            q = roll_p.tile([P, N], BF, tag="qslot", name="qs")
            k = roll_p.tile([P, N], BF, tag="kslot", name="ks")
            st[b].update(q=q, k=k)
            for i in range(NI):
                proj_q(b, st[b]["x"], q, i)
            for t0 in range(0, P * cs[b], FI):
                proj_k(b, st[b]["c"], k, t0)

        def alloc_v(b):
            v = roll_p.tile([P, 8, 2 * VW], BF, tag="vslot", name="vs")
            st[b]["v"] = v
            return v

        def stage_dma(b):
            st[b] = {"x": alloc_x(), "c": alloc_c()}
            dma_x(b, st[b]["x"])
            dma_c(b, st[b]["c"])

        # Prologue: first batch projections + second batch DMA.
        stage_qkv_qk(b0)
        alloc_v(b0)
        for j in range(cs[b0]):
            proj_v(b0, st[b0]["c"], st[b0]["v"], j)
        if B > 1:
            stage_dma(order[1])

        for pidx, b in enumerate(order):
            bn = order[pidx + 1] if pidx + 1 < B else None
            bp = order[pidx - 1] if pidx >= 1 else None
            bpp = order[pidx - 2] if pidx >= 2 else None
            bnn = order[pidx + 2] if pidx + 2 < B else None
            cat = roll_p.tile([P, 8, P], F32, tag="catslot", name="cat",
                              bufs=3)
            catT = roll_p.tile([P, N], BF, tag="catTslot", name="catT",
                               bufs=3)
            st[b].update(cat=cat, catT=catT)

            # 0. transposes of the previous batch (deps complete -> the
            # sync queue never blocks input DMAs behind them)
            if bp is not None:
                for g in range(8):
                    transpose_g(st[bp]["cat"], st[bp]["catT"], g)

            # 1. dots of b, interleaved with out-proj of b-2 (or, on the
            # first iteration, with q/k chains of the next batch)
            if bpp is not None:
                fillers = [
                    (lambda t=t, i2=i2, bb=bpp, ct=st[bpp]["catT"]:
                     final_group(bb, t, i2, ct))
                    for t in range(8) for i2 in range(NI)]
                qk_as_filler = False
            elif bn is not None:
                # first two iterations have no out-proj backlog: use the
                # next batch's q/k chains as the dots-region PE filler
                q = roll_p.tile([P, N], BF, tag="qslot", name="qs")
                kk = roll_p.tile([P, N], BF, tag="kslot", name="ks")
                st[bn].update(q=q, k=kk)
                fillers = [
                    (lambda i=i: proj_q(bn, st[bn]["x"], q, i))
                    for i in range(NI)] + [
                    (lambda t0=t0: proj_k(bn, st[bn]["c"], kk, t0))
                    for t0 in range(0, P * cs[bn], FI)]
                qk_as_filler = True
            else:
                fillers = []
                qk_as_filler = False
            dots_exp(b, 0, st[b]["q"], st[b]["k"], fillers)
            dots_exp(b, 1, st[b]["q"], st[b]["k"], fillers)

            # 2. q/k projections of the next batch
            if bn is not None and not qk_as_filler:
                stage_qkv_qk(bn)

            # 3. av of b interleaved with v-proj of the next batch and
            # any dots-region filler leftovers (small-c batches)
            if bn is not None:
                alloc_v(bn)
            last = pidx == B - 1
            for g in range(8):
                av_norm(b, g, st[b]["v"], cat, catT)
                if bn is not None and g < cs[bn]:
                    proj_v(bn, st[bn]["c"], st[bn]["v"], g)
                if fillers:
                    fillers.pop(0)()
                if last:
                    # fold the final two batches' out-proj into the last
                    # av region: transpose each cat tile as soon as its
                    # norm lands, then emit its out-proj groups
                    blp = order[-2]
                    transpose_g(cat, catT, g)
                    final_group(blp, g, 0, st[blp]["catT"], on_act=False)
                    final_group(blp, g, 1, st[blp]["catT"], on_act=True)
            while fillers:
                fillers.pop(0)()

            # 4. input DMAs for batch b+2
            if bnn is not None:
                stage_dma(bnn)
            if bpp is not None:
                st.pop(bpp)

        # Epilogue: out-proj of the last batch (catT transposed per-g
        # inside the last av region above).
        bl = order[-1]
        for t in range(8):
            for i2 in range(NI):
                final_group(bl, t, i2, st[bl]["catT"], on_act=(i2 == 1))

    nc.finalize()
    return nc


def _prep_shared(x, context, lengths, Wq, Wkv, Wout, bout):
    """Host-side prep shared across cores."""
    bf = ml_dtypes.bfloat16
    cs = tuple(int(min(N, (int(l) + P - 1) // P * P) // P) for l in lengths)
    shared = {}
    for b in range(B):
        shared[f"xT{b}"] = np.ascontiguousarray(
            np.asarray(x[b]).T, dtype=bf)
        w = P * cs[b]
        cb = np.asarray(context[b][:w]).copy()
        cb[int(lengths[b]):] = 0.0
        shared[f"cT{b}"] = np.ascontiguousarray(cb.T, dtype=bf)
    # mask: column ct = chunk j of batch b; 1.0 where key row valid
    ctot = sum(cs)
    maskb = np.zeros((P, ctot), dtype=np.float32)
    col = 0
    for b in range(B):
        for j in range(cs[b]):
            rows = np.arange(j * P, (j + 1) * P)
            maskb[:, col] = (rows < int(lengths[b])).astype(np.float32)
            col += 1
    shared["maskb"] = np.ascontiguousarray(maskb)
    shared["ident"] = np.ascontiguousarray(np.eye(P, dtype=np.float32))
    return cs, shared


def _prep_in_maps(x, context, lengths, Wq, Wkv, Wout, bout):
    bf = ml_dtypes.bfloat16
    cs, shared = _prep_shared(x, context, lengths, Wq, Wkv, Wout, bout)
    wq = np.asarray(Wq)
    wkv = np.asarray(Wkv)
    wout = np.asarray(Wout)
    in_maps = []
    for core in range(B):
        s0, s1 = core * P, (core + 1) * P
        m = dict(shared)
        m["wq_p"] = np.ascontiguousarray(wq[:, s0:s1], dtype=bf)
        m["wk_p"] = np.ascontiguousarray(wkv[:, s0:s1], dtype=bf)
        m["wv_p"] = np.ascontiguousarray(wkv[:, INNER + s0:INNER + s1], dtype=bf)
        m["wout_p"] = np.ascontiguousarray(wout[s0:s1, :], dtype=bf)
        in_maps.append(m)
    return cs, in_maps


def run(inputs: dict, trace: bool = False):
    cs, in_maps = _prep_in_maps(**inputs)
    if cs not in _CACHE:
        _CACHE[cs] = _build(cs)
    nc = _CACHE[cs]
    res = run_bass_kernel_spmd(nc, in_maps, core_ids=list(range(B)), trace=trace)
    bout = np.asarray(inputs["bout"], dtype=np.float32)
    out = np.empty((B, N, DIM), dtype=np.float32)
    for b in range(B):
        acc = np.zeros((N, DIM), dtype=np.float32)
        for core in range(B):
            acc += np.asarray(res.results[core][f"pout{b}"],
                              dtype=np.float32)
        out[b] = acc + bout
    return out, res


def kernel(**inputs) -> np.ndarray:
    out, _ = run(inputs, trace=False)
    return out
